# revision 1
# baseline (speedup 1.0000x reference)
"""Causal multi-head attention (B=2, S=2048, D=1024, H=16) on 8 trn2 cores.

Sharding: core c handles heads {2c, 2c+1} of BOTH batches (4 (b,h) pairs).
Per core:
  - project host-pretransposed x_b^T [D, S] (both batches) through the
    core's Wqkv column slice into Q^T/K^T head-pair tiles and V (natural
    layout, with a fused ones-column that makes the AV matmul emit softmax
    denominators),
  - causal attention per (batch, head) in transposed layout: scores^T =
    K Q^T chunks (PE row-tiled head pairs), exp on ScalarE, causal diagonal
    masks via gpsimd affine_select, A^T V on PE,
  - one 8-wide AllToAll redistributes head outputs so core c holds ALL 16
    heads of batch c//4 for sequence quarter c%4,
  - local projection through the full Wout emits final rows
    512*(c%4) .. +512 of batch c//4.
Host assembles the 8 [512, 1024] shards into (2, 2048, 1024).

Matmuls run in float32r (TF32-like single-pass PE mode, ~1e-3 rel err,
4x faster than true fp32). The PE rounds f32r inputs internally, so DRAM
inputs are declared float32r and DMA'd with the fast HW-DGE path with no
pre-rounding. Set _USE_F32R = False for full fp32.
"""

import sys

for _p in ("/opt/trn_rl_repo", "/opt/pypackages"):
    if _p not in sys.path:
        sys.path.insert(0, _p)

import numpy as np

import concourse.bass as bass
import concourse.mybir as mybir
import concourse.tile as tile
from concourse import bacc
from concourse.bass_utils import run_bass_kernel_spmd

B = 2
S = 2048
D = 1024
H = 16
DH = 64
NCORES = 8
SB = 512           # q block (matmul moving dim)
KC = 128           # k chunk (contraction tile)
NSB = S // SB      # 4 q-blocks
NKC = S // KC      # 16 k-chunks
NDC = D // KC      # 8 contraction chunks for the projections

_USE_F32R = True

_compiled = None


def _build():
    f32 = mybir.dt.float32
    bf16 = mybir.dt.bfloat16
    fr = mybir.dt.float32r if _USE_F32R else f32
    nc = bacc.Bacc(None, target_bir_lowering=False)

    # host-blocked inputs: every [128, N] tile is contiguous in DRAM.
    # Matmul inputs are declared float32r: same 4-byte data, PE rounds
    # internally, and plain (non-casting) sync DMA is allowed.
    xt = nc.declare_dram_parameter("xt", [B, NSB, NDC, KC, SB], fr, isOutput=False)
    wqk = nc.declare_dram_parameter("wqk", [NDC, KC, 2 * KC], fr, isOutput=False)
    wv = nc.declare_dram_parameter("wv", [NDC, KC, 2 * KC], fr, isOutput=False)
    wout = nc.declare_dram_parameter("wout", [NDC, KC, D], fr, isOutput=False)
    bqk = nc.declare_dram_parameter("bqk", [2 * KC], f32, isOutput=False)
    bv = nc.declare_dram_parameter("bv", [2 * DH], f32, isOutput=False)
    bo = nc.declare_dram_parameter("bo", [D], f32, isOutput=False)
    vones = nc.declare_dram_parameter("vones", [KC, NKC], fr, isOutput=False)
    out_ext = nc.declare_dram_parameter("out", [SB, D], f32, isOutput=True)

    # AllToAll staging: block t -> core t gets my heads of batch t//4 for
    # s-quarter t%4.
    a2a_in = nc.dram_tensor("a2a_in", [NCORES, KC, SB], fr)
    a2a_out = nc.dram_tensor("a2a_out", [NCORES, KC, SB], fr)

    with tile.TileContext(nc) as tc:
        with (
            tc.tile_pool(name="qkv", bufs=1) as qkvp,
            tc.tile_pool(name="obuf", bufs=1) as op,
            tc.tile_pool(name="misc", bufs=1) as mp,
            tc.tile_pool(name="evict", bufs=1) as ep,
        ):
            # ---- small constants -----------------------------------------
            bqk_t = [mp.tile([KC, 1], f32, tag=f"bqk{m}", name=f"bqk{m}")
                     for m in range(2)]
            for m in range(2):
                nc.scalar.dma_start(
                    out=bqk_t[m][:],
                    in_=bqk[m * KC:(m + 1) * KC].rearrange("(p o) -> p o", o=1),
                )
            bv_row = mp.tile([1, 2 * DH], f32, tag="bv_row")
            nc.scalar.dma_start(out=bv_row[:], in_=bv.rearrange("(o f) -> o f", o=1))
            bv_bc = mp.tile([KC, 2 * DH], f32, tag="bv_bc")
            nc.gpsimd.partition_broadcast(out_ap=bv_bc[:], in_ap=bv_row[:])
            bo_row = mp.tile([1, D], f32, tag="bo_row")
            nc.scalar.dma_start(out=bo_row[:], in_=bo.rearrange("(o f) -> o f", o=1))
            bo_bc = mp.tile([KC, D], f32, tag="bo_bc")
            nc.gpsimd.partition_broadcast(out_ap=bo_bc[:], in_ap=bo_row[:])

            # ---- persistent activations ----------------------------------
            # pair p = batch p with heads (2c, 2c+1).
            # QQ[p]: rows 0:64 = Q^T of head 2c, rows 64:128 = head 2c+1
            # per-sblk tiles so attention can start before all of the
            # projection finishes (Tile deps are per-tile)
            QQ = [[qkvp.tile([KC, SB], fr, tag=f"QQ{p}_{s}", name=f"QQ{p}_{s}")
                   for s in range(NSB)] for p in range(2)]
            KK = [[qkvp.tile([KC, SB], fr, tag=f"KK{p}_{s}", name=f"KK{p}_{s}")
                   for s in range(NSB)] for p in range(2)]
            # V[2p+hh][s]: [128, 4*65]; chunk sc at cols sc*65..+64; col 64: 1.0
            NCS = SB // KC
            V = [[qkvp.tile([KC, NCS * (DH + 1)], fr, tag=f"V{v}_{s}",
                            name=f"V{v}_{s}")
                  for s in range(NSB)] for v in range(4)]
            vones_sb = mp.tile([KC, NKC], fr, tag="vones_sb")
            nc.scalar.dma_start(out=vones_sb[:], in_=vones[:])
            for v in range(4):
                for s in range(NSB):
                    vv = V[v][s][:].rearrange("p (k c) -> p k c", c=DH + 1)
                    nc.vector.tensor_copy(
                        vv[:, :, DH], vones_sb[:, s * NCS:(s + 1) * NCS])
            # O[p]: rows 0:64 = head 2c out^T (normalized), 64:128 = head 2c+1
            O = [op.tile([KC, S], fr, tag=f"O{p}", name=f"O{p}") for p in range(2)]

            # ---- phase 1: projections ------------------------------------
            with (
                tc.tile_pool(name="pjw", bufs=1) as wp,
                tc.tile_pool(name="xbuf", bufs=24) as xp,
                tc.tile_pool(name="psum_proj", bufs=1, space="PSUM") as pp,
            ):
                wqk_t = [wp.tile([KC, 2 * KC], fr, tag=f"wqk{k}", name=f"wqk{k}")
                         for k in range(NDC)]
                wv_t = [wp.tile([KC, 2 * KC], fr, tag=f"wv{k}", name=f"wv{k}")
                        for k in range(NDC)]
                for k in range(NDC):
                    nc.sync.dma_start(out=wqk_t[k][:], in_=wqk[k])

                for sblk in range(NSB):
                    for bb in range(B):
                        xs = []
                        for k in range(NDC):
                            xtl = xp.tile([KC, SB], fr, tag="xt")
                            eng = nc.sync if k % 2 == 0 else nc.gpsimd
                            eng.dma_start(out=xtl[:], in_=xt[bb, sblk, k])
                            xs.append(xtl)
                        # m-chunk 0 -> QQ[bb], 1 -> KK[bb]
                        for m in range(2):
                            ps = pp.tile([KC, SB], f32, tag="ps_qk", bufs=4)
                            for k in range(NDC):
                                nc.tensor.matmul(
                                    ps[:],
                                    wqk_t[k][:, m * KC:(m + 1) * KC],
                                    xs[k][:],
                                    start=(k == 0),
                                    stop=(k == NDC - 1),
                                )
                            dest = (QQ if m == 0 else KK)[bb][sblk]
                            nc.vector.tensor_scalar_add(
                                dest[:], ps[:], bqk_t[m][:],
                            )
                        if sblk == 0 and bb == 0:
                            # defer Wv loads so the first QK matmuls (which
                            # need only wqk + x) start as early as possible
                            for k in range(NDC):
                                nc.gpsimd.dma_start(out=wv_t[k][:], in_=wv[k])
                        # V natural: lhsT = x^T chunk; rhs = Wv (zero-padded
                        # to N=256 so f32r streams at full rate)
                        for sc in range(SB // KC):
                            ps = pp.tile([KC, 2 * KC], f32, tag="ps_v", bufs=4)
                            for k in range(NDC):
                                nc.tensor.matmul(
                                    ps[:],
                                    xs[k][:, sc * KC:(sc + 1) * KC],
                                    wv_t[k][:],
                                    start=(k == 0),
                                    stop=(k == NDC - 1),
                                )
                            for hh in range(2):
                                nc.vector.tensor_add(
                                    V[2 * bb + hh][sblk][:, sc * (DH + 1):
                                                         sc * (DH + 1) + DH],
                                    ps[:, hh * DH:(hh + 1) * DH],
                                    bv_bc[:, hh * DH:(hh + 1) * DH],
                                )

            # ---- phase 2: attention --------------------------------------
            with (
                tc.tile_pool(name="pbuf", bufs=1) as pb,
                tc.tile_pool(name="psum_att", bufs=1, space="PSUM") as pa,
            ):
                for qblk in range(NSB):
                    nkc = 4 * (qblk + 1)  # causal: k-chunks 0..nkc-1
                    P_all = []
                    for p in range(B):
                        # P[kc]: [128, 1024]; cols hh*512.. hold head hh
                        P = [
                            pb.tile([KC, 2 * SB], fr, tag=f"P{kc}",
                                    name=f"P{kc}_{p}_{qblk}",
                                    bufs=(2 if kc < 11 else 1))
                            for kc in range(nkc)
                        ]
                        P_all.append(P)
                        for kc in range(nkc):
                            d = kc - 4 * qblk
                            # causal: columns < 128*d are fully masked; skip
                            # them in the matmul/exp where the speed holds up
                            c0 = min(KC * max(d, 0), 2 * KC)
                            ps = pa.tile([KC, 2 * SB], f32, tag="ps_s", bufs=3)
                            for hh in range(2):  # row-tiled head pair
                                r0 = hh * DH
                                nc.tensor.matmul(
                                    ps[:, hh * SB + c0:(hh + 1) * SB],
                                    KK[p][kc // 4][r0:r0 + DH,
                                                   (kc % 4) * KC:
                                                   (kc % 4 + 1) * KC],
                                    QQ[p][qblk][r0:r0 + DH, c0:SB],
                                    start=True,
                                    stop=True,
                                )
                            ps3 = ps[:].rearrange("p (h f) -> p h f", h=2)
                            pd3 = P[kc][:].rearrange("p (h f) -> p h f", h=2)
                            e0 = KC * max(d, 0)
                            nc.scalar.activation(
                                pd3[:, :, e0:SB],
                                ps3[:, :, e0:SB],
                                mybir.ActivationFunctionType.Exp,
                                scale=1.0 / float(np.sqrt(DH)),
                            )
                            if d >= 0:  # diagonal chunk: zero where k > q
                                # only columns >= c0 are ever read by the AV
                                # matmul, so mask just that range
                                nc.gpsimd.affine_select(
                                    out=pd3[:, :, c0:SB],
                                    in_=pd3[:, :, c0:SB],
                                    pattern=[[0, 2], [1, SB - c0]],
                                    compare_op=mybir.AluOpType.is_ge,
                                    fill=0.0,
                                    base=c0 - KC * d,
                                    channel_multiplier=-1,
                                )
                    for p in range(B):
                        P = P_all[p]
                        pos = [pa.tile([DH + 1, SB], f32, tag=f"ps_av{hh}",
                                       bufs=1, name=f"po{hh}_{p}_{qblk}")
                               for hh in range(2)]
                        for kc in range(nkc):
                            d = kc - 4 * qblk
                            c0 = min(KC * max(d, 0), 2 * KC)
                            for hh in range(2):
                                nc.tensor.matmul(
                                    pos[hh][:, c0:SB],
                                    V[2 * p + hh][kc // 4][:,
                                        (kc % 4) * (DH + 1):
                                        (kc % 4 + 1) * (DH + 1)],
                                    P[kc][:, hh * SB + c0:(hh + 1) * SB],
                                    start=(kc == 0),
                                    stop=(kc == nkc - 1),
                                )
                        for hh in range(2):
                            po = pos[hh]
                            # free the psum bank immediately; normalize later
                            avst = ep.tile([DH + 1, SB], f32, tag="avst", bufs=4)
                            nc.vector.tensor_copy(avst[:], po[:])
                            den0 = ep.tile([1, SB], f32, tag="den0", bufs=1)
                            nc.vector.tensor_copy(den0[:], avst[DH:DH + 1, :])
                            rden = ep.tile([1, SB], f32, tag="rden", bufs=1)
                            rscr = ep.tile([1, SB], f32, tag="rscr", bufs=1)
                            nc.vector.reciprocal_approx_accurate(
                                rden[:], den0[:], rscr[:])
                            rden_bc = ep.tile([DH, SB], f32, tag="rden_bc", bufs=2)
                            nc.gpsimd.partition_broadcast(
                                out_ap=rden_bc[:], in_ap=rden[:]
                            )
                            r0 = hh * DH
                            nc.vector.tensor_mul(
                                O[p][r0:r0 + DH, qblk * SB:(qblk + 1) * SB],
                                avst[0:DH, :],
                                rden_bc[:],
                            )
                        # stage this (batch, quarter) block for the AllToAll
                        nc.sync.dma_start(
                            out=a2a_in[4 * p + qblk],
                            in_=O[p][:, qblk * SB:(qblk + 1) * SB],
                        )

            # ---- phase 3: head exchange + output projection --------------
            nc.gpsimd.collective_compute(
                "AllToAll",
                mybir.AluOpType.bypass,
                replica_groups=[[0, 1, 2, 3, 4, 5, 6, 7]],
                ins=[a2a_in[:]],
                outs=[a2a_out[:]],
            )
            with (
                tc.tile_pool(name="wout_pool", bufs=1) as wop,
                tc.tile_pool(name="recv", bufs=1) as rp,
                tc.tile_pool(name="psum_out", bufs=1, space="PSUM") as pu,
            ):
                wout_t = [wop.tile([KC, D], fr, tag=f"wo{k}", name=f"wo{k}")
                          for k in range(NDC)]
                for k in range(NDC):
                    nc.sync.dma_start(out=wout_t[k][:], in_=wout[k])
                # a2a_out block i = heads (2i, 2i+1) of my batch for my
                # quarter -> flat [1024, 512] = attnout^T in global head order
                recv = [rp.tile([KC, SB], fr, tag=f"rc{k}", name=f"rc{k}")
                        for k in range(NDC)]
                for k in range(NDC):
                    eng = nc.sync if k % 2 == 0 else nc.gpsimd
                    eng.dma_start(out=recv[k][:], in_=a2a_out[k])
                for sc in range(SB // KC):
                    for nb in range(D // SB):
                        ps = pu.tile([KC, SB], f32, tag="ps_o", bufs=4)
                        for k in range(NDC):
                            nc.tensor.matmul(
                                ps[:],
                                recv[k][:, sc * KC:(sc + 1) * KC],
                                wout_t[k][:, nb * SB:(nb + 1) * SB],
                                start=(k == 0),
                                stop=(k == NDC - 1),
                            )
                        ot = ep.tile([KC, SB], f32, tag="osb", bufs=4)
                        nc.vector.tensor_add(
                            ot[:], ps[:], bo_bc[:, nb * SB:(nb + 1) * SB]
                        )
                        nc.sync.dma_start(
                            out=out_ext[sc * KC:(sc + 1) * KC,
                                        nb * SB:(nb + 1) * SB],
                            in_=ot[:],
                        )

    nc.compile()
    return nc


def _get_program():
    global _compiled
    if _compiled is None:
        _compiled = _build()
    return _compiled


def _shard_inputs(x, Wqkv, bqkv, Wout, bout):
    """Build the 8 per-core input maps (all host-side numpy)."""
    x = np.ascontiguousarray(x, dtype=np.float32)
    Wqkv = np.asarray(Wqkv, dtype=np.float32)
    bqkv = np.asarray(bqkv, dtype=np.float32)
    Wout = np.asarray(Wout, dtype=np.float32)
    bout = np.ascontiguousarray(np.asarray(bout, dtype=np.float32))

    Wq = Wqkv[:, 0 * D:1 * D]
    Wk = Wqkv[:, 1 * D:2 * D]
    Wv_full = Wqkv[:, 2 * D:3 * D]
    bq = bqkv[0 * D:1 * D]
    bk = bqkv[1 * D:2 * D]
    bv_full = bqkv[2 * D:3 * D]

    # shared across all cores
    xt = np.ascontiguousarray(
        x.transpose(0, 2, 1)                      # [B, D, S]
         .reshape(B, D, NSB, SB).transpose(0, 2, 1, 3)
         .reshape(B, NSB, NDC, KC, SB)
    )
    wout_b = np.ascontiguousarray(Wout.reshape(NDC, KC, D))
    vones = np.ones((KC, NKC), dtype=np.float32)

    in_maps = []
    for c in range(NCORES):
        ha, hb = 2 * c, 2 * c + 1
        wqk_c = np.ascontiguousarray(np.concatenate(
            [Wq[:, ha * DH:(ha + 1) * DH], Wq[:, hb * DH:(hb + 1) * DH],
             Wk[:, ha * DH:(ha + 1) * DH], Wk[:, hb * DH:(hb + 1) * DH]],
            axis=1).reshape(NDC, KC, 2 * KC))
        bqk_c = np.ascontiguousarray(np.concatenate(
            [bq[ha * DH:(ha + 1) * DH], bq[hb * DH:(hb + 1) * DH],
             bk[ha * DH:(ha + 1) * DH], bk[hb * DH:(hb + 1) * DH]]))
        # Wv zero-padded to 256 columns so the V matmul moving dim is 256
        wv_c = np.zeros((D, 2 * KC), dtype=np.float32)
        wv_c[:, 0:DH] = Wv_full[:, ha * DH:(ha + 1) * DH]
        wv_c[:, DH:2 * DH] = Wv_full[:, hb * DH:(hb + 1) * DH]
        wv_c = np.ascontiguousarray(wv_c.reshape(NDC, KC, 2 * KC))
        bv_c = np.ascontiguousarray(np.concatenate(
            [bv_full[ha * DH:(ha + 1) * DH], bv_full[hb * DH:(hb + 1) * DH]]))
        in_maps.append({
            "xt": xt, "wqk": wqk_c, "wv": wv_c, "wout": wout_b,
            "bqk": bqk_c, "bv": bv_c, "bo": bout, "vones": vones,
        })
    return in_maps


def run(inputs, trace=False, trace_kwargs=None):
    nc = _get_program()
    in_maps = _shard_inputs(**inputs)
    res = run_bass_kernel_spmd(
        nc, in_maps, list(range(NCORES)), trace=trace,
        **(trace_kwargs or {}),
    )
    out = np.empty((B, S, D), dtype=np.float32)
    for c in range(NCORES):
        b = c // 4
        r0 = SB * (c % 4)
        out[b, r0:r0 + SB, :] = res.results[c]["out"]
    return out, res


def kernel(**inputs):
    out, _ = run(inputs)
    return out



# revision 8
# speedup vs baseline: 1.2963x; 1.2963x over previous
"""Causal multi-head attention (B=2, S=2048, D=1024, H=16) on 8 trn2 cores.

Sharding: core c computes heads {2c, 2c+1} of BOTH batches (tensor parallel
over heads). All matmul operands are bf16 (psum accumulation fp32).

Pipeline: 4 stages (stage s = seq quarter s). Stage s emits, interleaved so
the PE queue never head-blocks on ScalarE exp:
  - attention for q-block s (scores^T = K Q^T row-tiled head pairs on PE,
    exp on ScalarE, narrow causal band masks on gpsimd, A^T V on PE with a
    fused ones-column emitting softmax denominators),
  - projection MM chains for seq quarter s+1 (Q^T/K^T transposed layout,
    V natural layout packed 4-chunks-per-psum-bank),
  - output projection for quarter s-1 (after that quarter's AllToAll).
After each stage: per-quarter 8-way AllToAll (256KB/core) redistributes
head outputs so EVERY core gets one 128-row strip of each quarter
(out-projection load spread evenly; only the last quarter's a2a+proj is
exposed). Host assembles the 8 x 4 strips into (2, 2048, 1024).
"""

import sys

for _p in ("/opt/trn_rl_repo", "/opt/pypackages"):
    if _p not in sys.path:
        sys.path.insert(0, _p)

import ml_dtypes
import numpy as np

import concourse.bass as bass
import concourse.mybir as mybir
import concourse.tile as tile
from concourse import bacc
from concourse.bass_utils import run_bass_kernel_spmd

B = 2
S = 2048
D = 1024
H = 16
DH = 64
NCORES = 8
SB = 512           # q block (matmul moving dim)
KC = 128           # k chunk (contraction tile)
NSB = S // SB      # 4 q-blocks / seq quarters
NKC = S // KC      # 16 k-chunks
NDC = D // KC      # 8 contraction chunks for the projections

BF16 = ml_dtypes.bfloat16

_compiled = None


def _interleave(main, fillers, late_fillers=()):
    """Emission-order weave: spread `fillers` evenly among `main` units,
    `late_fillers` evenly through the second half."""
    n = max(len(main), 1)
    slots = [[] for _ in range(n + 1)]
    nf = len(fillers)
    for i, f in enumerate(fillers):
        slots[min(((i + 1) * n) // (nf + 1), n)].append(f)
    nl = len(late_fillers)
    for i, f in enumerate(late_fillers):
        pos = n // 2 + ((i + 1) * (n - n // 2)) // (nl + 1)
        slots[min(pos, n)].append(f)
    for i, m in enumerate(main):
        for f in slots[i]:
            f()
        m()
    for f in slots[n]:
        f()


def _build():
    f32 = mybir.dt.float32
    bf16 = mybir.dt.bfloat16
    nc = bacc.Bacc(None, target_bir_lowering=False)

    # host-blocked inputs: every [128, N] tile is contiguous in DRAM.
    xt = nc.declare_dram_parameter("xt", [B, NSB, NDC, KC, SB], bf16, isOutput=False)
    wqk = nc.declare_dram_parameter("wqk", [NDC, KC, 2 * KC], bf16, isOutput=False)
    wv = nc.declare_dram_parameter("wv", [NDC, KC, 2 * DH], bf16, isOutput=False)
    wout = nc.declare_dram_parameter("wout", [NDC, KC, D], bf16, isOutput=False)
    bqk = nc.declare_dram_parameter("bqk", [2 * KC], f32, isOutput=False)
    bv4 = nc.declare_dram_parameter("bv4", [4 * 2 * DH], f32, isOutput=False)
    bo = nc.declare_dram_parameter("bo", [D], f32, isOutput=False)
    out_ext = nc.declare_dram_parameter("out", [NSB, KC, D], f32, isOutput=True)

    # Per-quarter AllToAll staging. Block i of quarter q = (batch i//4,
    # seq rows 512q + 128*(i%4)): core i ends up owning that 128-row strip.
    a2a_in = [nc.dram_tensor(f"a2a_in{q}", [NCORES, KC, KC], bf16)
              for q in range(NSB)]
    a2a_out = [nc.dram_tensor(f"a2a_out{q}", [NCORES, KC, KC], bf16)
               for q in range(NSB)]

    with tile.TileContext(nc) as tc:
        with (
            tc.tile_pool(name="wts", bufs=1) as wp,
            tc.tile_pool(name="qkv", bufs=1) as qkvp,
            tc.tile_pool(name="xbuf", bufs=20) as xp,
            tc.tile_pool(name="pbuf", bufs=1) as pb,
            tc.tile_pool(name="obuf", bufs=1) as op,
            tc.tile_pool(name="recv", bufs=1) as rp,
            tc.tile_pool(name="misc", bufs=1) as mp,
            tc.tile_pool(name="evict", bufs=1) as ep,
            tc.tile_pool(name="psum_pj", bufs=1, space="PSUM") as pp,
            tc.tile_pool(name="psum_sc", bufs=1, space="PSUM") as pa,
            tc.tile_pool(name="psum_av", bufs=1, space="PSUM") as pv,
        ):
            # ---- small constants -----------------------------------------
            bqk_t = [mp.tile([KC, 1], f32, tag=f"bqk{m}", name=f"bqk{m}")
                     for m in range(2)]
            for m in range(2):
                nc.sync.dma_start(
                    out=bqk_t[m][:],
                    in_=bqk[m * KC:(m + 1) * KC].rearrange("(p o) -> p o", o=1),
                )
            bv_row = mp.tile([1, 4 * 2 * DH], f32, tag="bv_row")
            nc.sync.dma_start(out=bv_row[:], in_=bv4.rearrange("(o f) -> o f", o=1))
            bv_bc4 = mp.tile([KC, 4 * 2 * DH], f32, tag="bv_bc4")
            nc.gpsimd.partition_broadcast(out_ap=bv_bc4[:], in_ap=bv_row[:])
            bo_row = mp.tile([1, D], f32, tag="bo_row")
            nc.sync.dma_start(out=bo_row[:], in_=bo.rearrange("(o f) -> o f", o=1))
            bo_bc = mp.tile([KC, D], f32, tag="bo_bc")
            nc.gpsimd.partition_broadcast(out_ap=bo_bc[:], in_ap=bo_row[:])

            # ---- persistent weights --------------------------------------
            wqk_t = [wp.tile([KC, 2 * KC], bf16, tag=f"wqk{k}", name=f"wqk{k}")
                     for k in range(NDC)]
            wv_t = [wp.tile([KC, 2 * DH], bf16, tag=f"wv{k}", name=f"wv{k}")
                    for k in range(NDC)]
            wout_t = [wp.tile([KC, D], bf16, tag=f"wo{k}", name=f"wo{k}")
                      for k in range(NDC)]
            for k in range(NDC):
                nc.sync.dma_start(out=wqk_t[k][:], in_=wqk[k])
            for k in range(NDC):
                nc.gpsimd.dma_start(out=wv_t[k][:], in_=wv[k])

            # ---- persistent activations ----------------------------------
            # QQ[p][s]: rows 0:64 = Q^T of head 2c, rows 64:128 = head 2c+1
            QQ = [[qkvp.tile([KC, SB], bf16, tag=f"QQ{p}_{s}", name=f"QQ{p}_{s}")
                   for s in range(NSB)] for p in range(2)]
            KK = [[qkvp.tile([KC, SB], bf16, tag=f"KK{p}_{s}", name=f"KK{p}_{s}")
                   for s in range(NSB)] for p in range(2)]
            # V[2p+hh][s]: [128, 4*65]; chunk sc at cols sc*65..+64; col 64: 1.0
            NCS = SB // KC
            V = [[qkvp.tile([KC, NCS * (DH + 1)], bf16, tag=f"V{v}_{s}",
                            name=f"V{v}_{s}")
                  for s in range(NSB)] for v in range(4)]
            for v in range(4):
                for s in range(NSB):
                    vv = V[v][s][:].rearrange("p (k c) -> p k c", c=DH + 1)
                    nc.vector.memset(vv[:, :, DH:DH + 1], 1.0)

            # ---- unit builders -------------------------------------------
            def load_xt(sblk):
                xs = []
                for bb in range(B):
                    row = []
                    for k in range(NDC):
                        xtl = xp.tile([KC, SB], bf16, tag="xt", name=f"x{sblk}_{bb}_{k}")
                        eng = nc.sync if k % 2 == 0 else nc.gpsimd
                        eng.dma_start(out=xtl[:], in_=xt[bb, sblk, k])
                        row.append(xtl)
                    xs.append(row)
                return xs

            def proj_units(sblk, xs):
                """6 PE chain units projecting seq quarter `sblk`."""
                units = []
                for bb in range(B):
                    for m in range(2):
                        def qk_unit(bb=bb, m=m):
                            ps = pp.tile([KC, SB], f32, tag="ps_pj", bufs=2)
                            for k in range(NDC):
                                nc.tensor.matmul(
                                    ps[:],
                                    wqk_t[k][:, m * KC:(m + 1) * KC],
                                    xs[bb][k][:],
                                    start=(k == 0),
                                    stop=(k == NDC - 1),
                                )
                            dest = (QQ if m == 0 else KK)[bb][sblk]
                            nc.vector.tensor_scalar_add(dest[:], ps[:], bqk_t[m][:])
                        units.append(qk_unit)

                    def v_unit(bb=bb):
                        # natural layout: lhsT = x^T chunk (stationary),
                        # rhs = Wv [128, 128]; 4 sc regions in one psum bank
                        psv = pp.tile([KC, SB], f32, tag="ps_pj", bufs=2)
                        for sc in range(NCS):
                            for k in range(NDC):
                                nc.tensor.matmul(
                                    psv[:, sc * KC:(sc + 1) * KC],
                                    xs[bb][k][:, sc * KC:(sc + 1) * KC],
                                    wv_t[k][:],
                                    start=(k == 0),
                                    stop=(k == NDC - 1),
                                )
                        ps3 = psv[:].rearrange("p (k c) -> p k c", c=2 * DH)
                        bv3 = bv_bc4[:].rearrange("p (k c) -> p k c", c=2 * DH)
                        for hh in range(2):
                            vd = V[2 * bb + hh][sblk][:].rearrange(
                                "p (k c) -> p k c", c=DH + 1)
                            nc.vector.tensor_add(
                                vd[:, :, 0:DH],
                                ps3[:, :, hh * DH:(hh + 1) * DH],
                                bv3[:, :, hh * DH:(hh + 1) * DH],
                            )
                    units.append(v_unit)
                return units

            def outproj_units(q):
                """recv + 2 PE chain units projecting my strip of quarter q."""
                recv = rp.tile([KC, NCORES * KC], bf16, tag="recv", bufs=2,
                               name=f"recv{q}")

                def recv_unit():
                    nc.sync.dma_start(
                        out=recv[:], in_=a2a_out[q][:].rearrange("s p i -> p s i"))

                units = [recv_unit]
                for nb in range(2):
                    def o_unit(nb=nb):
                        ps = pp.tile([KC, SB], f32, tag="ps_pj", bufs=2)
                        for k in range(NDC):
                            nc.tensor.matmul(
                                ps[:],
                                recv[:, k * KC:(k + 1) * KC],
                                wout_t[k][:, nb * SB:(nb + 1) * SB],
                                start=(k == 0),
                                stop=(k == NDC - 1),
                            )
                        ot = ep.tile([KC, SB], f32, tag="ot", bufs=4)
                        nc.vector.tensor_add(
                            ot[:], ps[:], bo_bc[:, nb * SB:(nb + 1) * SB])
                        nc.sync.dma_start(
                            out=out_ext[q, :, nb * SB:(nb + 1) * SB], in_=ot[:])
                    units.append(o_unit)
                return units

            def attn_units(qblk):
                """Attention chunk units for q-block `qblk`, both batches.
                Per chunk: scores pair (row-tiled head pair) -> exp -> mask;
                AV matmuls woven 2 chunks behind; tail: normalize + a2a
                staging."""
                nkc = 4 * (qblk + 1)
                units = []
                for p in range(2):
                    P = [pb.tile([KC, 2 * SB], bf16, tag=f"P{kc}",
                                 name=f"P{kc}_{p}_{qblk}",
                                 bufs=(2 if kc < 12 else 1))
                         for kc in range(nkc)]
                    pos = [pv.tile([DH + 1, SB], f32, tag=f"pos{hh}",
                                   name=f"pos{hh}_{p}_{qblk}", bufs=1)
                           for hh in range(2)]

                    def av(p, kc, d, P=P, pos=pos, nkc=nkc):
                        c0 = KC * max(d, 0)
                        for hh in range(2):
                            nc.tensor.matmul(
                                pos[hh][:, c0:SB],
                                V[2 * p + hh][kc // 4][:, (kc % 4) * (DH + 1):
                                                       (kc % 4 + 1) * (DH + 1)],
                                P[kc][:, hh * SB + c0:(hh + 1) * SB],
                                start=(kc == 0),
                                stop=(kc == nkc - 1),
                            )

                    for kc in range(nkc):
                        def chunk(p=p, kc=kc, pos=pos, P=P, av=av):
                            d = kc - 4 * qblk
                            c0 = KC * max(d, 0)
                            ps = pa.tile([KC, 2 * SB], f32, tag="ps_s", bufs=2)
                            for hh in range(2):  # row-tiled head pair
                                r0 = hh * DH
                                nc.tensor.matmul(
                                    ps[:, hh * SB + c0:(hh + 1) * SB],
                                    KK[p][kc // 4][r0:r0 + DH,
                                                   (kc % 4) * KC:(kc % 4 + 1) * KC],
                                    QQ[p][qblk][r0:r0 + DH, c0:SB],
                                    start=True,
                                    stop=True,
                                )
                            ps3 = ps[:].rearrange("p (h f) -> p h f", h=2)
                            pd3 = P[kc][:].rearrange("p (h f) -> p h f", h=2)
                            nc.scalar.activation(
                                pd3[:, :, c0:SB],
                                ps3[:, :, c0:SB],
                                mybir.ActivationFunctionType.Exp,
                                scale=1.0 / float(np.sqrt(DH)),
                            )
                            if d >= 0:  # diagonal chunk: zero band where k > q
                                nc.gpsimd.affine_select(
                                    out=pd3[:, :, c0:c0 + KC],
                                    in_=pd3[:, :, c0:c0 + KC],
                                    pattern=[[0, 2], [1, KC]],
                                    compare_op=mybir.AluOpType.is_ge,
                                    fill=0.0,
                                    base=0,
                                    channel_multiplier=-1,
                                )
                            if kc >= 2:
                                av(p, kc - 2, kc - 2 - 4 * qblk)
                        units.append(chunk)

                    def tail(p=p, pos=pos, av=av, nkc=nkc):
                        av(p, nkc - 2, nkc - 2 - 4 * qblk)
                        av(p, nkc - 1, nkc - 1 - 4 * qblk)
                        O = op.tile([KC, SB], bf16, tag=f"O{p}", bufs=2,
                                    name=f"O{p}_{qblk}")
                        for hh in range(2):
                            den = ep.tile([1, SB], f32, tag="den", bufs=2)
                            nc.vector.tensor_copy(den[:], pos[hh][DH:DH + 1, :])
                            rden = ep.tile([1, SB], f32, tag="rden", bufs=2)
                            nc.vector.reciprocal_approx_fast(
                                out=rden[:], in_=den[:])
                            rden_bc = ep.tile([DH, SB], f32, tag="rbc", bufs=2)
                            nc.gpsimd.partition_broadcast(
                                out_ap=rden_bc[:], in_ap=rden[:])
                            nc.vector.tensor_mul(
                                O[hh * DH:(hh + 1) * DH, :],
                                pos[hh][0:DH, :],
                                rden_bc[:],
                            )
                        nc.sync.dma_start(
                            out=a2a_in[qblk][4 * p:4 * p + 4].rearrange(
                                "j p i -> p j i"),
                            in_=O[:].rearrange("p (j i) -> p j i", i=KC),
                        )
                    units.append(tail)
                return units

            # ---- the pipeline --------------------------------------------
            xs = load_xt(0)
            for u in proj_units(0, xs):
                u()
            # wout not needed until out-projection of quarter 0 (stage 1);
            # issue after the first projections so it doesn't delay them
            for k in range(NDC):
                nc.sync.dma_start(out=wout_t[k][:], in_=wout[k])
            for stage in range(NSB):
                fillers = []
                late = []
                if stage < NSB - 1:
                    xs = load_xt(stage + 1)
                    fillers = proj_units(stage + 1, xs)
                if stage >= 1:
                    late = outproj_units(stage - 1)
                _interleave(attn_units(stage), fillers, late)
                nc.gpsimd.collective_compute(
                    "AllToAll",
                    mybir.AluOpType.bypass,
                    replica_groups=[[0, 1, 2, 3, 4, 5, 6, 7]],
                    ins=[a2a_in[stage][:]],
                    outs=[a2a_out[stage][:]],
                )
            for u in outproj_units(NSB - 1):
                u()

    nc.compile()
    return nc


def _get_program():
    global _compiled
    if _compiled is None:
        _compiled = _build()
    return _compiled


def _shard_inputs(x, Wqkv, bqkv, Wout, bout):
    """Build the 8 per-core input maps (all host-side numpy, bf16 data)."""
    x = np.asarray(x, dtype=np.float32)
    Wqkv = np.asarray(Wqkv, dtype=np.float32)
    bqkv = np.asarray(bqkv, dtype=np.float32)
    Wout = np.asarray(Wout, dtype=np.float32)
    bout = np.ascontiguousarray(np.asarray(bout, dtype=np.float32))

    Wq = Wqkv[:, 0 * D:1 * D]
    Wk = Wqkv[:, 1 * D:2 * D]
    Wv_full = Wqkv[:, 2 * D:3 * D]
    bq = bqkv[0 * D:1 * D]
    bk = bqkv[1 * D:2 * D]
    bv_full = bqkv[2 * D:3 * D]

    # shared across all cores
    xt = np.ascontiguousarray(
        x.transpose(0, 2, 1)                      # [B, D, S]
         .reshape(B, NDC, KC, NSB, SB).transpose(0, 3, 1, 2, 4)
    ).astype(BF16)
    wout_b = np.ascontiguousarray(Wout.reshape(NDC, KC, D)).astype(BF16)

    in_maps = []
    for c in range(NCORES):
        ha, hb = 2 * c, 2 * c + 1
        wqk_c = np.ascontiguousarray(np.concatenate(
            [Wq[:, ha * DH:(ha + 1) * DH], Wq[:, hb * DH:(hb + 1) * DH],
             Wk[:, ha * DH:(ha + 1) * DH], Wk[:, hb * DH:(hb + 1) * DH]],
            axis=1).reshape(NDC, KC, 2 * KC)).astype(BF16)
        bqk_c = np.ascontiguousarray(np.concatenate(
            [bq[ha * DH:(ha + 1) * DH], bq[hb * DH:(hb + 1) * DH],
             bk[ha * DH:(ha + 1) * DH], bk[hb * DH:(hb + 1) * DH]]))
        wv_c = np.ascontiguousarray(np.concatenate(
            [Wv_full[:, ha * DH:(ha + 1) * DH],
             Wv_full[:, hb * DH:(hb + 1) * DH]],
            axis=1).reshape(NDC, KC, 2 * DH)).astype(BF16)
        bv_c = np.concatenate(
            [bv_full[ha * DH:(ha + 1) * DH], bv_full[hb * DH:(hb + 1) * DH]])
        bv4_c = np.ascontiguousarray(np.tile(bv_c, SB // KC))
        in_maps.append({
            "xt": xt, "wqk": wqk_c, "wv": wv_c, "wout": wout_b,
            "bqk": bqk_c, "bv4": bv4_c, "bo": bout,
        })
    return in_maps


def run(inputs, trace=False, trace_kwargs=None):
    nc = _get_program()
    in_maps = _shard_inputs(**inputs)
    res = run_bass_kernel_spmd(
        nc, in_maps, list(range(NCORES)), trace=trace,
        **(trace_kwargs or {}),
    )
    out = np.empty((B, S, D), dtype=np.float32)
    for c in range(NCORES):
        b = c // 4
        for q in range(NSB):
            r0 = SB * q + KC * (c % 4)
            out[b, r0:r0 + KC, :] = res.results[c]["out"][q]
    return out, res


def kernel(**inputs):
    out, _ = run(inputs)
    return out


# revision 9
# speedup vs baseline: 1.5042x; 1.1604x over previous
"""Causal multi-head attention (B=2, S=2048, D=1024, H=16) on 8 trn2 cores.

Sharding: core c computes heads {2c, 2c+1} of BOTH batches (tensor parallel
over heads). All matmul operands are bf16 (psum accumulation fp32).

Pipeline: 4 stages (stage s = seq quarter s). Stage s emits, interleaved so
the PE queue never head-blocks on ScalarE exp:
  - attention for q-block s (scores^T = K Q^T row-tiled head pairs on PE,
    exp on ScalarE, narrow causal band masks on gpsimd, A^T V on PE with a
    fused ones-column emitting softmax denominators),
  - projection MM chains for seq quarter s+1 (Q^T/K^T transposed layout,
    V natural layout packed 4-chunks-per-psum-bank),
  - output projection for quarter s-1 (after that quarter's AllToAll).
After each stage: per-quarter 8-way AllToAll (256KB/core) redistributes
head outputs so EVERY core gets one 128-row strip of each quarter
(out-projection load spread evenly; only the last quarter's a2a+proj is
exposed). Host assembles the 8 x 4 strips into (2, 2048, 1024).
"""

import sys

for _p in ("/opt/trn_rl_repo", "/opt/pypackages"):
    if _p not in sys.path:
        sys.path.insert(0, _p)

import ml_dtypes
import numpy as np

import concourse.bass as bass
import concourse.mybir as mybir
import concourse.tile as tile
from concourse import bacc
from concourse.bass_utils import run_bass_kernel_spmd

B = 2
S = 2048
D = 1024
H = 16
DH = 64
NCORES = 8
SB = 512           # q block (matmul moving dim)
KC = 128           # k chunk (contraction tile)
NSB = S // SB      # 4 q-blocks / seq quarters
NKC = S // KC      # 16 k-chunks
NDC = D // KC      # 8 contraction chunks for the projections

BF16 = ml_dtypes.bfloat16

_compiled = None


def _interleave(main, fillers, late_fillers=()):
    """Emission-order weave: spread `fillers` evenly among `main` units,
    `late_fillers` evenly through the second half."""
    n = max(len(main), 1)
    slots = [[] for _ in range(n + 1)]
    nf = len(fillers)
    for i, f in enumerate(fillers):
        slots[min(((i + 1) * n) // (nf + 1), n)].append(f)
    nl = len(late_fillers)
    for i, f in enumerate(late_fillers):
        pos = n // 2 + ((i + 1) * (n - n // 2)) // (nl + 1)
        slots[min(pos, n)].append(f)
    for i, m in enumerate(main):
        for f in slots[i]:
            f()
        m()
    for f in slots[n]:
        f()


def _build():
    f32 = mybir.dt.float32
    bf16 = mybir.dt.bfloat16
    nc = bacc.Bacc(None, target_bir_lowering=False)

    # host-blocked inputs: every [128, N] tile is contiguous in DRAM.
    xt = nc.declare_dram_parameter("xt", [B, NSB, NDC, KC, SB], bf16, isOutput=False)
    wqk = nc.declare_dram_parameter("wqk", [NDC, KC, 2 * KC], bf16, isOutput=False)
    wv = nc.declare_dram_parameter("wv", [NDC, KC, 2 * DH], bf16, isOutput=False)
    wout = nc.declare_dram_parameter("wout", [NDC, KC, D], bf16, isOutput=False)
    bqk = nc.declare_dram_parameter("bqk", [2 * KC], f32, isOutput=False)
    bv4 = nc.declare_dram_parameter("bv4", [4 * 2 * DH], f32, isOutput=False)
    bo = nc.declare_dram_parameter("bo", [D], f32, isOutput=False)
    out_ext = nc.declare_dram_parameter("out", [NSB, KC, D], f32, isOutput=True)

    # Per-quarter AllToAll staging. Block i of quarter q = (batch i//4,
    # seq rows 512q + 128*(i%4)): core i ends up owning that 128-row strip.
    a2a_in = [nc.dram_tensor(f"a2a_in{q}", [NCORES, KC, KC], bf16)
              for q in range(NSB)]
    a2a_out = [nc.dram_tensor(f"a2a_out{q}", [NCORES, KC, KC], bf16)
               for q in range(NSB)]

    with tile.TileContext(nc) as tc:
        with (
            tc.tile_pool(name="wts", bufs=1) as wp,
            tc.tile_pool(name="qkv", bufs=1) as qkvp,
            tc.tile_pool(name="xbuf", bufs=20) as xp,
            tc.tile_pool(name="pbuf", bufs=1) as pb,
            tc.tile_pool(name="obuf", bufs=1) as op,
            tc.tile_pool(name="recv", bufs=1) as rp,
            tc.tile_pool(name="misc", bufs=1) as mp,
            tc.tile_pool(name="evict", bufs=1) as ep,
            tc.tile_pool(name="psum_pj", bufs=1, space="PSUM") as pp,
            tc.tile_pool(name="psum_sc", bufs=1, space="PSUM") as pa,
            tc.tile_pool(name="psum_av", bufs=1, space="PSUM") as pv,
        ):
            # ---- small constants -----------------------------------------
            bqk_t = [mp.tile([KC, 1], f32, tag=f"bqk{m}", name=f"bqk{m}")
                     for m in range(2)]
            for m in range(2):
                nc.sync.dma_start(
                    out=bqk_t[m][:],
                    in_=bqk[m * KC:(m + 1) * KC].rearrange("(p o) -> p o", o=1),
                )
            bv_row = mp.tile([1, 4 * 2 * DH], f32, tag="bv_row")
            nc.sync.dma_start(out=bv_row[:], in_=bv4.rearrange("(o f) -> o f", o=1))
            bv_bc4 = mp.tile([KC, 4 * 2 * DH], f32, tag="bv_bc4")
            nc.gpsimd.partition_broadcast(out_ap=bv_bc4[:], in_ap=bv_row[:])
            bo_row = mp.tile([1, D], f32, tag="bo_row")
            nc.sync.dma_start(out=bo_row[:], in_=bo.rearrange("(o f) -> o f", o=1))
            bo_bc = mp.tile([KC, D], f32, tag="bo_bc")
            nc.gpsimd.partition_broadcast(out_ap=bo_bc[:], in_ap=bo_row[:])

            # ---- persistent weights --------------------------------------
            wqk_t = [wp.tile([KC, 2 * KC], bf16, tag=f"wqk{k}", name=f"wqk{k}")
                     for k in range(NDC)]
            wv_t = [wp.tile([KC, 2 * DH], bf16, tag=f"wv{k}", name=f"wv{k}")
                    for k in range(NDC)]
            wout_t = [wp.tile([KC, D], bf16, tag=f"wo{k}", name=f"wo{k}")
                      for k in range(NDC)]
            for k in range(NDC):
                nc.sync.dma_start(out=wqk_t[k][:], in_=wqk[k])
            for k in range(NDC):
                nc.gpsimd.dma_start(out=wv_t[k][:], in_=wv[k])

            # ---- persistent activations ----------------------------------
            # QQ[p][s]: rows 0:64 = Q^T of head 2c, rows 64:128 = head 2c+1
            QQ = [[qkvp.tile([KC, SB], bf16, tag=f"QQ{p}_{s}", name=f"QQ{p}_{s}")
                   for s in range(NSB)] for p in range(2)]
            KK = [[qkvp.tile([KC, SB], bf16, tag=f"KK{p}_{s}", name=f"KK{p}_{s}")
                   for s in range(NSB)] for p in range(2)]
            # V[2p+hh][s]: [128, 4*65]; chunk sc at cols sc*65..+64; col 64: 1.0
            NCS = SB // KC
            V = [[qkvp.tile([KC, NCS * (DH + 1)], bf16, tag=f"V{v}_{s}",
                            name=f"V{v}_{s}")
                  for s in range(NSB)] for v in range(4)]
            for v in range(4):
                for s in range(NSB):
                    vv = V[v][s][:].rearrange("p (k c) -> p k c", c=DH + 1)
                    nc.vector.memset(vv[:, :, DH:DH + 1], 1.0)

            # ---- unit builders -------------------------------------------
            def load_xt(sblk):
                xs = []
                for bb in range(B):
                    row = []
                    for k in range(NDC):
                        xtl = xp.tile([KC, SB], bf16, tag="xt", name=f"x{sblk}_{bb}_{k}")
                        eng = nc.sync if k % 2 == 0 else nc.gpsimd
                        eng.dma_start(out=xtl[:], in_=xt[bb, sblk, k])
                        row.append(xtl)
                    xs.append(row)
                return xs

            def proj_units(sblk, xs):
                """6 PE chain units projecting seq quarter `sblk`."""
                units = []
                for bb in range(B):
                    for m in range(2):
                        def qk_unit(bb=bb, m=m):
                            ps = pp.tile([KC, SB], f32, tag="ps_pj", bufs=2)
                            for k in range(NDC):
                                nc.tensor.matmul(
                                    ps[:],
                                    wqk_t[k][:, m * KC:(m + 1) * KC],
                                    xs[bb][k][:],
                                    start=(k == 0),
                                    stop=(k == NDC - 1),
                                )
                            dest = (QQ if m == 0 else KK)[bb][sblk]
                            nc.vector.tensor_scalar_add(dest[:], ps[:], bqk_t[m][:])
                        units.append(qk_unit)

                    def v_unit(bb=bb):
                        # natural layout: lhsT = x^T chunk (stationary),
                        # rhs = Wv [128, 128]; 4 sc regions in one psum bank
                        psv = pp.tile([KC, SB], f32, tag="ps_pj", bufs=2)
                        for sc in range(NCS):
                            for k in range(NDC):
                                nc.tensor.matmul(
                                    psv[:, sc * KC:(sc + 1) * KC],
                                    xs[bb][k][:, sc * KC:(sc + 1) * KC],
                                    wv_t[k][:],
                                    start=(k == 0),
                                    stop=(k == NDC - 1),
                                )
                        ps3 = psv[:].rearrange("p (k c) -> p k c", c=2 * DH)
                        bv3 = bv_bc4[:].rearrange("p (k c) -> p k c", c=2 * DH)
                        for hh in range(2):
                            vd = V[2 * bb + hh][sblk][:].rearrange(
                                "p (k c) -> p k c", c=DH + 1)
                            nc.vector.tensor_add(
                                vd[:, :, 0:DH],
                                ps3[:, :, hh * DH:(hh + 1) * DH],
                                bv3[:, :, hh * DH:(hh + 1) * DH],
                            )
                    units.append(v_unit)
                return units

            def outproj_units(q):
                """recv + 2 PE chain units projecting my strip of quarter q."""
                recv = rp.tile([KC, NCORES * KC], bf16, tag="recv", bufs=2,
                               name=f"recv{q}")

                def recv_unit():
                    nc.sync.dma_start(
                        out=recv[:], in_=a2a_out[q][:].rearrange("s p i -> p s i"))

                units = [recv_unit]
                for nb in range(2):
                    def o_unit(nb=nb):
                        ps = pp.tile([KC, SB], f32, tag="ps_pj", bufs=2)
                        for k in range(NDC):
                            nc.tensor.matmul(
                                ps[:],
                                recv[:, k * KC:(k + 1) * KC],
                                wout_t[k][:, nb * SB:(nb + 1) * SB],
                                start=(k == 0),
                                stop=(k == NDC - 1),
                            )
                        ot = ep.tile([KC, SB], f32, tag="ot", bufs=4)
                        nc.vector.tensor_add(
                            ot[:], ps[:], bo_bc[:, nb * SB:(nb + 1) * SB])
                        nc.sync.dma_start(
                            out=out_ext[q, :, nb * SB:(nb + 1) * SB], in_=ot[:])
                    units.append(o_unit)
                return units

            # ---- attention machinery -------------------------------------
            # P/pos tiles created lazily at emission time so pool tag
            # rotation follows true program order (qblk-3 p0 chunks kc<8 are
            # emitted early, during stage 2, and get dedicated tags).
            P_reg = {}
            pos_reg = {}

            def get_P(qblk, p, kc):
                key = (qblk, p, kc)
                if key not in P_reg:
                    if qblk == 3 and p == 0 and kc < 8:
                        tag, bufs = f"P3e_{kc}", 1
                    else:
                        tag, bufs = f"P{kc}", (2 if kc < 12 else 1)
                    P_reg[key] = pb.tile([KC, 2 * SB], bf16, tag=tag,
                                         bufs=bufs, name=f"P_{qblk}_{p}_{kc}")
                return P_reg[key]

            def get_pos(qblk, p, hh):
                key = (qblk, p, hh)
                if key not in pos_reg:
                    pos_reg[key] = pv.tile([DH + 1, SB], f32, tag=f"pos{hh}",
                                           bufs=1, name=f"pos{hh}_{p}_{qblk}")
                return pos_reg[key]

            def chunk_unit(qblk, p, kc):
                def u():
                    d = kc - 4 * qblk
                    c0 = KC * max(d, 0)
                    P = get_P(qblk, p, kc)
                    ps = pa.tile([KC, 2 * SB], f32, tag="ps_s", bufs=2)
                    for hh in range(2):  # row-tiled head pair
                        r0 = hh * DH
                        nc.tensor.matmul(
                            ps[:, hh * SB + c0:(hh + 1) * SB],
                            KK[p][kc // 4][r0:r0 + DH,
                                           (kc % 4) * KC:(kc % 4 + 1) * KC],
                            QQ[p][qblk][r0:r0 + DH, c0:SB],
                            start=True,
                            stop=True,
                        )
                    ps3 = ps[:].rearrange("p (h f) -> p h f", h=2)
                    pd3 = P[:].rearrange("p (h f) -> p h f", h=2)
                    nc.scalar.activation(
                        pd3[:, :, c0:SB],
                        ps3[:, :, c0:SB],
                        mybir.ActivationFunctionType.Exp,
                        scale=1.0 / float(np.sqrt(DH)),
                    )
                    if d >= 0:  # diagonal chunk: zero band where k > q
                        nc.gpsimd.affine_select(
                            out=pd3[:, :, c0:c0 + KC],
                            in_=pd3[:, :, c0:c0 + KC],
                            pattern=[[0, 2], [1, KC]],
                            compare_op=mybir.AluOpType.is_ge,
                            fill=0.0,
                            base=0,
                            channel_multiplier=-1,
                        )
                return u

            def av_emit(qblk, p, kc):
                d = kc - 4 * qblk
                c0 = KC * max(d, 0)
                nkc = 4 * (qblk + 1)
                P = get_P(qblk, p, kc)
                for hh in range(2):
                    nc.tensor.matmul(
                        get_pos(qblk, p, hh)[:, c0:SB],
                        V[2 * p + hh][kc // 4][:, (kc % 4) * (DH + 1):
                                               (kc % 4 + 1) * (DH + 1)],
                        P[:, hh * SB + c0:(hh + 1) * SB],
                        start=(kc == 0),
                        stop=(kc == nkc - 1),
                    )

            def tail_unit(qblk, p):
                def u():
                    O = op.tile([KC, SB], bf16, tag=f"O{p}", bufs=2,
                                name=f"O{p}_{qblk}")
                    for hh in range(2):
                        pos = get_pos(qblk, p, hh)
                        den = ep.tile([1, SB], f32, tag="den", bufs=2)
                        nc.vector.tensor_copy(den[:], pos[DH:DH + 1, :])
                        rden = ep.tile([1, SB], f32, tag="rden", bufs=2)
                        nc.vector.reciprocal_approx_fast(out=rden[:], in_=den[:])
                        rden_bc = ep.tile([DH, SB], f32, tag="rbc", bufs=2)
                        nc.gpsimd.partition_broadcast(
                            out_ap=rden_bc[:], in_ap=rden[:])
                        nc.vector.tensor_mul(
                            O[hh * DH:(hh + 1) * DH, :],
                            pos[0:DH, :],
                            rden_bc[:],
                        )
                    nc.sync.dma_start(
                        out=a2a_in[qblk][4 * p:4 * p + 4].rearrange(
                            "j p i -> p j i"),
                        in_=O[:].rearrange("p (j i) -> p j i", i=KC),
                    )
                return u

            def attn_p_units(qblk, p, early_done=()):
                """Chunk units (minus early_done) with AV matmuls woven in at
                a matched rate (AV lags 2 chunks so exp stays ahead), then the
                normalize/staging tail."""
                nkc = 4 * (qblk + 1)
                chunks = [kc for kc in range(nkc) if kc not in early_done]
                units = []
                ai = 0
                for i, kc in enumerate(chunks):
                    units.append(chunk_unit(qblk, p, kc))
                    target = ((i + 1) * nkc) // len(chunks)
                    emit = []
                    while ai < min(target, nkc) and (
                            ai in early_done or ai <= kc - 2):
                        emit.append(ai)
                        ai += 1
                    if emit:
                        def avs(emit=emit):
                            for a in emit:
                                av_emit(qblk, p, a)
                        units.append(avs)

                def drain(ai0=ai):
                    for a in range(ai0, nkc):
                        av_emit(qblk, p, a)
                units.append(drain)
                units.append(tail_unit(qblk, p))
                return units

            # ---- the pipeline --------------------------------------------
            # stage s: attention for q-block s, interleaved with projection
            # chains for quarter s+1 and (from stage 2) the out-projection of
            # quarter s-2 (one extra stage of slack absorbs a2a peer skew).
            # Stage 2 additionally pre-runs scores+exp for qblk3/p0 kc<8 to
            # level ScalarE load between stages 2 and 3.
            EARLY3 = tuple(range(8))
            xs = load_xt(0)
            for u in proj_units(0, xs):
                u()
            for stage in range(NSB):
                mains = []
                for p in range(2):
                    early = EARLY3 if (stage == 3 and p == 0) else ()
                    mains += attn_p_units(stage, p, early)
                if stage == 2:
                    mains += [chunk_unit(3, 0, kc) for kc in EARLY3]
                fillers = []
                late = []
                if stage < NSB - 1:
                    xs = load_xt(stage + 1)
                    fillers = proj_units(stage + 1, xs)
                if stage == 1:
                    def wout_loader():
                        for k in range(NDC):
                            nc.sync.dma_start(out=wout_t[k][:], in_=wout[k])
                    fillers = [wout_loader] + fillers
                if stage >= 2:
                    late = outproj_units(stage - 2)
                _interleave(mains, fillers, late)
                nc.gpsimd.collective_compute(
                    "AllToAll",
                    mybir.AluOpType.bypass,
                    replica_groups=[[0, 1, 2, 3, 4, 5, 6, 7]],
                    ins=[a2a_in[stage][:]],
                    outs=[a2a_out[stage][:]],
                )
            for q in (NSB - 2, NSB - 1):
                for u in outproj_units(q):
                    u()

    nc.compile()
    return nc


def _get_program():
    global _compiled
    if _compiled is None:
        _compiled = _build()
    return _compiled


def _shard_inputs(x, Wqkv, bqkv, Wout, bout):
    """Build the 8 per-core input maps (all host-side numpy, bf16 data)."""
    x = np.asarray(x, dtype=np.float32)
    Wqkv = np.asarray(Wqkv, dtype=np.float32)
    bqkv = np.asarray(bqkv, dtype=np.float32)
    Wout = np.asarray(Wout, dtype=np.float32)
    bout = np.ascontiguousarray(np.asarray(bout, dtype=np.float32))

    Wq = Wqkv[:, 0 * D:1 * D]
    Wk = Wqkv[:, 1 * D:2 * D]
    Wv_full = Wqkv[:, 2 * D:3 * D]
    bq = bqkv[0 * D:1 * D]
    bk = bqkv[1 * D:2 * D]
    bv_full = bqkv[2 * D:3 * D]

    # shared across all cores
    xt = np.ascontiguousarray(
        x.transpose(0, 2, 1)                      # [B, D, S]
         .reshape(B, NDC, KC, NSB, SB).transpose(0, 3, 1, 2, 4)
    ).astype(BF16)
    wout_b = np.ascontiguousarray(Wout.reshape(NDC, KC, D)).astype(BF16)

    in_maps = []
    for c in range(NCORES):
        ha, hb = 2 * c, 2 * c + 1
        wqk_c = np.ascontiguousarray(np.concatenate(
            [Wq[:, ha * DH:(ha + 1) * DH], Wq[:, hb * DH:(hb + 1) * DH],
             Wk[:, ha * DH:(ha + 1) * DH], Wk[:, hb * DH:(hb + 1) * DH]],
            axis=1).reshape(NDC, KC, 2 * KC)).astype(BF16)
        bqk_c = np.ascontiguousarray(np.concatenate(
            [bq[ha * DH:(ha + 1) * DH], bq[hb * DH:(hb + 1) * DH],
             bk[ha * DH:(ha + 1) * DH], bk[hb * DH:(hb + 1) * DH]]))
        wv_c = np.ascontiguousarray(np.concatenate(
            [Wv_full[:, ha * DH:(ha + 1) * DH],
             Wv_full[:, hb * DH:(hb + 1) * DH]],
            axis=1).reshape(NDC, KC, 2 * DH)).astype(BF16)
        bv_c = np.concatenate(
            [bv_full[ha * DH:(ha + 1) * DH], bv_full[hb * DH:(hb + 1) * DH]])
        bv4_c = np.ascontiguousarray(np.tile(bv_c, SB // KC))
        in_maps.append({
            "xt": xt, "wqk": wqk_c, "wv": wv_c, "wout": wout_b,
            "bqk": bqk_c, "bv4": bv4_c, "bo": bout,
        })
    return in_maps


def run(inputs, trace=False, trace_kwargs=None):
    nc = _get_program()
    in_maps = _shard_inputs(**inputs)
    res = run_bass_kernel_spmd(
        nc, in_maps, list(range(NCORES)), trace=trace,
        **(trace_kwargs or {}),
    )
    out = np.empty((B, S, D), dtype=np.float32)
    for c in range(NCORES):
        b = c // 4
        for q in range(NSB):
            r0 = SB * q + KC * (c % 4)
            out[b, r0:r0 + KC, :] = res.results[c]["out"][q]
    return out, res


def kernel(**inputs):
    out, _ = run(inputs)
    return out
